# revision 6
# baseline (speedup 1.0000x reference)
"""HardAttention kernel for Trainium2 (8 NeuronCores, Bass/Tile).

reference:
    scores = einsum("btd,bcsd->btcs", xs, ys)   # (B,Tx,C,Ty)
    out    = scores.max(-1).sum(1)              # (B,C)

Shapes: B=16, Tx=128, C=64, Ty=128, d=768.

Strategy:
  - Data-parallel over B: core i handles batches [2i, 2i+2).
  - Host pre-arranges both operands d-major so the PE (which contracts
    over the partition axis) can consume them directly:
        xsT[b, k, dk, t] = xs[b, t, 128k+dk]          (B, 6, 128, Tx)
        ysT[b, k, dk, c, s] = ys[b, c, s, 128k+dk]    (B, 6, 128, C, Ty)
  - Per (b, quarter-of-16-candidates): one 6 MB contiguous-ish DMA, then
    6 (k) x 4 (groups of 4 candidates -> N=512) accumulating matmuls into
    4 PSUM banks; DVE reduce_max over Ty per candidate into an SBUF tile
    M[t, c]; finally a ones-vector matmul contracts the partition axis
    (sum over t) -> out[b, c].
"""

import numpy as np

B, TX, C, TY, D = 16, 128, 64, 128, 768
N_CORES = 8
BPC = B // N_CORES          # batches per core = 2
KC = D // 128               # contraction chunks = 6
QC = 16                     # candidates per quarter (DMA slab)
NQ = C // QC                # quarters = 4
G = 4                       # candidates per matmul (N = G*TY = 512)

# Matmul operand dtype: "float32" (exact, 4 cyc/row) or "float32r"
# (reduced-precision multiply, 1 cyc/row at N>=512).
MM_DTYPE = "float32"

_CACHE = {}


def _build():
    import concourse.bass as bass
    import concourse.mybir as mybir
    import concourse.tile as tile
    from concourse import bacc

    mm_dt = getattr(mybir.dt, MM_DTYPE)
    f32 = mybir.dt.float32

    nc = bacc.Bacc(
        "TRN2",
        target_bir_lowering=False,
        debug=False,
        num_devices=N_CORES,
    )

    xs_ap = nc.dram_tensor("xsT", (BPC, KC, 128, TX), mm_dt, kind="ExternalInput").ap()
    ys_ap = nc.dram_tensor(
        "ysT", (BPC, KC, 128, C, TY), mm_dt, kind="ExternalInput"
    ).ap()
    out_ap = nc.dram_tensor("out", (BPC, C), f32, kind="ExternalOutput").ap()

    with tile.TileContext(nc) as tc:
        with (
            tc.tile_pool(name="xt", bufs=1) as xpool,
            tc.tile_pool(name="yt", bufs=3) as ypool,
            tc.tile_pool(name="mt", bufs=2) as mpool,
            tc.tile_pool(name="ones", bufs=1) as opool,
            tc.tile_pool(name="osb", bufs=2) as obpool,
            tc.tile_pool(name="ps", bufs=6, space="PSUM") as pspool,
            tc.tile_pool(name="pso", bufs=2, space="PSUM") as psopool,
        ):
            # All of xsT for this core: (dk, b, k, t)
            xt = xpool.tile([128, BPC, KC, TX], mm_dt)
            nc.sync.dma_start(xt[:], xs_ap.rearrange("b k p t -> p b k t"))

            ones = opool.tile([128, 1], f32)
            nc.any.memset(ones[:], 1.0)

            for b in range(BPC):
                m_b = mpool.tile([128, C], f32)  # max_s scores, [t, c]
                for q in range(NQ):
                    # slab: (dk, k, c_in_quarter, s)
                    yt = ypool.tile([128, KC, QC, TY], mm_dt)
                    nc.sync.dma_start(
                        yt[:],
                        ys_ap[b, :, :, q * QC : (q + 1) * QC, :].rearrange(
                            "k p c s -> p k c s"
                        ),
                    )
                    psums = [
                        pspool.tile([128, G, TY], f32, name=f"ps_{b}_{q}_{g}", tag="ps")
                        for g in range(G)
                    ]
                    for k in range(KC):
                        for g in range(G):
                            nc.tensor.matmul(
                                psums[g][:],
                                lhsT=xt[:, b, k, :],
                                rhs=yt[:, k, g * G : (g + 1) * G, :],
                                start=(k == 0),
                                stop=(k == KC - 1),
                            )
                    for g in range(G):
                        nc.vector.reduce_max(
                            m_b[:, q * QC + g * G : q * QC + (g + 1) * G],
                            psums[g][:],
                            axis=mybir.AxisListType.X,
                        )
                # sum over t (partition axis) via ones-vector matmul
                out_ps = psopool.tile([1, C], f32, tag="out_ps")
                nc.tensor.matmul(
                    out_ps[:], lhsT=ones[:], rhs=m_b[:], start=True, stop=True
                )
                osb = obpool.tile([1, C], f32, tag="osb")
                nc.vector.tensor_copy(osb[:], out_ps[:])
                nc.sync.dma_start(out_ap[b : b + 1, :], osb[:])

    nc.compile()
    return nc


def _get_nc():
    if "nc" not in _CACHE:
        _CACHE["nc"] = _build()
    return _CACHE["nc"]


def _prep(xs: np.ndarray, ys: np.ndarray):
    """Host-side layout: d-major, blocked by 128-chunks of d."""
    xs = np.ascontiguousarray(xs, dtype=np.float32)
    ys = np.ascontiguousarray(ys, dtype=np.float32)
    # xsT[b, k, dk, t] = xs[b, t, 128k+dk]
    xsT = np.ascontiguousarray(xs.transpose(0, 2, 1)).reshape(B, KC, 128, TX)
    # ysT[b, k, dk, c, s] = ys[b, c, s, 128k+dk]; per-b 2D transpose
    ysT = np.empty((B, KC, 128, C, TY), dtype=np.float32)
    flat_view = ysT.reshape(B, D, C * TY)
    for b in range(B):
        np.copyto(flat_view[b], ys[b].reshape(C * TY, D).T)
    return xsT, ysT


def kernel(xs: np.ndarray, ys: np.ndarray) -> np.ndarray:
    from concourse.bass_utils import run_bass_kernel_spmd

    nc = _get_nc()
    xsT, ysT = _prep(xs, ys)
    in_maps = [
        {
            "xsT": xsT[i * BPC : (i + 1) * BPC],
            "ysT": ysT[i * BPC : (i + 1) * BPC],
        }
        for i in range(N_CORES)
    ]
    res = run_bass_kernel_spmd(nc, in_maps, core_ids=list(range(N_CORES)))
    out = np.concatenate([res.results[i]["out"] for i in range(N_CORES)], axis=0)
    return out.astype(np.float32)
